# revision 46
# baseline (speedup 1.0000x reference)
"""Trainium2 Bass kernel for nn_DecoderHead (B=2, T=2048, D=1024, H=16, DH=64).

y = x + softmax_causal((x @ Wq.T) split to heads @ k^T / sqrt(D)) @ v

Sharding: 8 cores = 2 (batch) x 4 (head groups of 4 heads). Each core computes
its batch's q-projection for its 256 output features (Wq column-sharded by
head), causal attention for its 4 heads, adds the residual slice, and writes a
[T, 256] slice; the host concatenates slices (the all-gather over the
head-split d dim is a free host-side assembly).

Per-core dataflow (all matmul contractions on the PE partition axis; bf16
operands by default — matmul column rate is the same as fp32r but DMA and
DVE elementwise costs halve/quarter):
  qT[e, t]   = sum_d WqT[d, e] * xT[d, t]         (q projection, transposed)
  sT[tk, tq] = sum_dh kT_h[dh, tk] * qT_h[dh, tq] (scores, transposed; two
               heads run concurrently in distinct PE row-groups since DH=64)
  eT         = exp(sT / 32) * tri_mask            (ACT exp; both exp and the
               downstream PV matmuls are sliced to skip fully-masked query
               columns of diagonal key blocks; the triangle mask-mul touches
               only the [128,128] diagonal sub-block)
  oT[dh', tq]= sum_tk vO[tk, dh'] * eT[tk, tq]    (vO = [v | ones]; row 64
                                                   accumulates the denominator)
  y[tq, dh]  = transpose(oT) / denom + x_res      (PE transpose into one PSUM
                                                   bank, fused DVE epilogue)

Schedule is tq-tile-major and fully interleaved: load stage c+1, project q for
tile c, then run attention for tile c — attention for tile 0 starts after
~2.5 MB of DMA and overlaps the remaining loads.
"""

import os
from collections import deque

import numpy as np

import concourse.bass as bass
import concourse.mybir as mybir
import concourse.tile as tile
from concourse import bacc
from concourse.alu_op_type import AluOpType
from concourse.bass_utils import run_bass_kernel_spmd

# Problem shape (hardcoded per the harness contract).
B, T, D, H = 2, 2048, 1024, 16
DH = D // H          # 64
N_CORES = 8
HPC = H // (N_CORES // B)   # heads per core = 4
EPC = HPC * DH       # output features per core = 256
P = 128              # SBUF partitions
TQ = 512             # query-tile width (matmul moving-dim)
NTQ = T // TQ        # 4
NTKB = T // P        # 16 key blocks of 128
DT = D // P          # 8 contraction tiles for the q projection
EG = EPC // P        # 2 head-pair groups of 128 e-rows
SCALE = 1.0 / np.sqrt(np.float32(D))   # 1/32 (reference scales by sqrt(d))

F32 = mybir.dt.float32
I16 = mybir.dt.int16
BF16 = mybir.dt.bfloat16

# Schraudolph-style exp on the DVE: intN(s*KS + bias) bitcast as a float
# approximates exp(s*SCALE) (exponent exact, mantissa linearly interpolated,
# ~±3% sawtooth — softmax normalization cancels it to ~0.3% output error).
# The bias TILE folds the causal mask: masked entries get a tiny exponent
# (~2^-95, i.e. weight 0). Diagonal score blocks use this path so the ACT
# engine (the exp bottleneck) only handles off-diagonal blocks.
def _schrau_consts(variant):
    if variant == "bf16":
        # int16 bitcast as bf16: 2^7 exponent scale
        k = float(SCALE) * float(np.log2(np.e)) * 128.0
        return mybir.dt.int16, k, 127.0 * 128.0 - 7.41, 4096.0
    # int32 bitcast as fp32/fp32r: 2^23 exponent scale
    k = float(SCALE) * float(np.log2(np.e)) * float(1 << 23)
    return mybir.dt.int32, k, float(127 * (1 << 23) - 486411), 260.0e6

# Matmul operand dtype: fp32r (default; fp32 w/ 11-bit mantissa, full PE
# rate at moving>=256, no separate Ldweights on HW), bf16 (half DMA but HW
# pays explicit weight loads — measured slower), fp32 (exact, 1/4 rate).
VARIANT = os.environ.get("DH_VARIANT", "fp32r")


def _mm_dt(variant):
    return {
        "fp32": mybir.dt.float32,
        "fp32r": mybir.dt.float32r,
        "bf16": mybir.dt.bfloat16,
    }[variant]


def _np_round_fp32r(a: np.ndarray) -> np.ndarray:
    """Round fp32 to the fp32r value set: 11-bit mantissa, RNE, low 12 bits 0."""
    u = a.astype(np.float32).view(np.uint32)
    lsb = (u >> np.uint32(12)) & np.uint32(1)
    r = (u + np.uint32(0x7FF) + lsb) & np.uint32(0xFFFFF000)
    return r.view(np.float32)


def _host_cast(a: np.ndarray, variant: str) -> np.ndarray:
    a = np.ascontiguousarray(a, dtype=np.float32)
    if variant == "fp32r":
        return _np_round_fp32r(a)
    if variant == "bf16":
        import ml_dtypes
        return a.astype(ml_dtypes.bfloat16)
    return a


def build_nc(variant: str = VARIANT, repeat: int = 1):
    """Build the per-core SPMD Bass program. `repeat` wraps the body in a
    hardware loop (timing only)."""
    mdt = _mm_dt(variant)
    nc = bacc.Bacc(
        "TRN2", target_bir_lowering=False, debug=False, num_devices=N_CORES
    )

    # Layouts are partition-major so every DMA lands as 128 descriptors of
    # multi-KB contiguous runs (the SP sequencer pays ~7ns/descriptor).
    xT = nc.dram_tensor("xT", [P, NTQ, DT, TQ], mdt, kind="ExternalInput").ap()
    wqT = nc.dram_tensor("wqT", [P, DT, EPC], mdt, kind="ExternalInput").ap()
    kT = nc.dram_tensor("kT", [P, EG, T], mdt, kind="ExternalInput").ap()
    vO = nc.dram_tensor("vO", [P, NTKB, HPC, DH + 1], mdt, kind="ExternalInput").ap()
    xres = nc.dram_tensor("xres", [P, T // P, EPC], mdt, kind="ExternalInput").ap()
    maskT = nc.dram_tensor("maskT", [P, P], mdt, kind="ExternalInput").ap()
    ident = nc.dram_tensor("ident", [P, P], F32, kind="ExternalInput").ap()
    y = nc.dram_tensor("y", [P, NTQ, HPC, 4, DH], F32,
                       kind="ExternalOutput").ap()

    with tile.TileContext(nc) as tc:
        with (
            tc.tile_pool(name="const", bufs=1) as cpool,
            tc.tile_pool(name="xq", bufs=1) as xqpool,
            tc.tile_pool(name="work", bufs=6) as wpool,
            tc.tile_pool(name="epi", bufs=2) as epool,
            tc.tile_pool(name="ps_s", bufs=3, space="PSUM") as ps_s,
            tc.tile_pool(name="ps_o", bufs=2, space="PSUM") as ps_o,
        ):
            idt, ks_k, bs_v, bs_m = _schrau_consts(variant)

            def body(_iv=None):
                # ---- tiles -------------------------------------------------
                id_sb = cpool.tile([P, P], F32, name="id_sb", tag="id_sb")
                mk_sb = cpool.tile([P, P], mdt, name="mk_sb", tag="mk_sb")
                wq_sb = xqpool.tile([P, DT, EPC], mdt, name="wq_sb", tag="wq_sb")
                xT_sb = xqpool.tile([P, NTQ, DT, TQ], mdt, name="xT_sb",
                                    tag="xT_sb")
                kT_sb = cpool.tile([P, EG, T], mdt, name="kT_sb", tag="kT_sb")
                vO_sb = cpool.tile([P, NTKB, HPC, DH + 1], mdt, name="vO_sb",
                                   tag="vO_sb")
                xr_sb = cpool.tile([P, T // P, EPC], mdt, name="xr_sb",
                                   tag="xr_sb")
                qT_sb = xqpool.tile([P, EG, T], mdt, name="qT_sb", tag="qT_sb")

                # ---- loads ------------------------------------------------
                # xT streams per-stage (qproj for tile c needs stage c); all
                # other inputs are single whole-tensor DMAs (128 descriptors
                # each) issued upfront and overlapped by the DMA engines.
                nc.sync.dma_start(id_sb[:], ident[:])
                nc.sync.dma_start(wq_sb[:], wqT[:])

                def load_stage(c):
                    nc.sync.dma_start(xT_sb[:, c], xT[:, c])

                load_stage(NTQ - 1)
                nc.sync.dma_start(mk_sb[:], maskT[:])
                # Quarter the big constants so the first attention pairs
                # only wait on the head of each stream.
                for q in range(4):
                    tsl = slice(q * (T // 4), (q + 1) * (T // 4))
                    nc.sync.dma_start(kT_sb[:, :, tsl], kT[:, :, tsl])
                    nc.sync.dma_start(
                        vO_sb[:, 4 * q:4 * (q + 1)], vO[:, 4 * q:4 * (q + 1)]
                    )

                # Warm-up while stage-0 DMA streams: prime the ACT exp table
                # and keep PE busy so the HAM clock-gate opens (dummy work on
                # the identity tile; results unused).
                warm_et = wpool.tile([P, P], F32, name="warm_et", tag="warm")
                psw = ps_o.tile([P, P], F32, name="psw", tag="o")
                for w in range(12):
                    nc.tensor.matmul(
                        psw[:], id_sb[:], id_sb[:], start=True, stop=True,
                    )
                nc.scalar.activation(
                    warm_et[:], psw[:],
                    mybir.ActivationFunctionType.Exp, scale=0.01,
                )

                pending = deque()

                def epilogue_start(h, tqt, pso_t):
                    oT = epool.tile([DH + 1, TQ], F32, name="oT", tag="oT",
                                    bufs=4)
                    nc.vector.tensor_copy(oT[:], pso_t[:])
                    return (h, tqt, oT)

                def epilogue(state):
                    h, tqt, oT = state
                    ysb = epool.tile([P, 4, DH], F32, name="ysb", tag="ysb",
                                     bufs=4)
                    pst = ps_o.tile([P, 4, DH + 1], F32, name="pst", tag="o")
                    for j in range(4):
                        nc.tensor.transpose(
                            pst[:, j, :],
                            oT[:, j * P:(j + 1) * P],
                            id_sb[0:DH + 1, 0:DH + 1],
                        )
                    rc = epool.tile([P, 4], F32, name="rc", tag="rc", bufs=4)
                    nc.vector.reciprocal(rc[:], pst[:, :, DH])
                    for j in range(4):
                        nc.vector.scalar_tensor_tensor(
                            ysb[:, j, :],
                            pst[:, j, 0:DH],
                            rc[:, j:j + 1],
                            xr_sb[:, 4 * tqt + j, h * DH:(h + 1) * DH],
                            AluOpType.mult,
                            AluOpType.add,
                        )
                    nc.sync.dma_start(y[:, tqt, h], ysb[:])

                def attention(hp, tqt):
                    g = hp
                    ntk = 4 * (tqt + 1)
                    npairs = ntk // 2
                    tq0 = tqt * TQ

                    def start_col(tkb):
                        """First non-fully-masked query column (within the
                        tile) for key block tkb."""
                        m = tkb - 4 * tqt
                        return P * m if m > 0 else 0

                    pso2 = [
                        ps_o.tile([DH + 1, TQ], F32, name=f"pso{i}", tag="o")
                        for i in range(2)
                    ]

                    def emit_pv(p_et2, p_pair, last=False):
                        for u in range(2):
                            tkb = 2 * p_pair + u
                            s0 = start_col(tkb)
                            for i in range(2):
                                nc.tensor.matmul(
                                    pso2[i][:, s0:],
                                    vO_sb[:, tkb, 2 * hp + i, :],
                                    p_et2[i][:, u, s0:],
                                    start=(tkb == 0),
                                    stop=(last and u == 1),
                                    skip_group_check=True,
                                )

                    prev = None
                    for pair in range(npairs):
                        diag = 2 * pair >= 4 * tqt
                        et2 = []
                        pssc2 = [
                            ps_s.tile([P, 2, TQ], F32, name=f"pssc{i}", tag="s")
                            for i in range(2)
                        ]
                        for i in range(2):
                            for u in range(2):
                                tkb = 2 * pair + u
                                s0 = start_col(tkb)
                                bp = DH * i
                                nc.tensor.matmul(
                                    pssc2[i][:, u, s0:],
                                    kT_sb[bp:bp + DH, g,
                                          tkb * P:(tkb + 1) * P],
                                    qT_sb[bp:bp + DH, g, tq0 + s0:tq0 + TQ],
                                    start=True,
                                    stop=True,
                                )
                        for i in range(2):
                            et = wpool.tile([P, 2, TQ], mdt,
                                            name=f"et{i}", tag="et")
                            if diag:
                                for u in range(2):
                                    tkb = 2 * pair + u
                                    s0 = start_col(tkb)
                                    nc.scalar.activation(
                                        et[:, u, s0:], pssc2[i][:, u, s0:],
                                        mybir.ActivationFunctionType.Exp,
                                        scale=float(SCALE),
                                    )
                                    nc.vector.tensor_mul(
                                        et[:, u, s0:s0 + P],
                                        et[:, u, s0:s0 + P],
                                        mk_sb[:],
                                    )
                            else:
                                nc.scalar.activation(
                                    et[:], pssc2[i][:],
                                    mybir.ActivationFunctionType.Exp,
                                    scale=float(SCALE),
                                )
                            et2.append(et)
                        if prev is not None:
                            emit_pv(*prev)
                        prev = (et2, pair)
                        if pending and pair < 2:
                            epilogue(pending.popleft())
                    emit_pv(*prev, last=True)
                    for i in range(2):
                        pending.append(epilogue_start(2 * hp + i, tqt, pso2[i]))

                def qproj(tqc):
                    sl = bass.ts(tqc, TQ)
                    for g in range(EG):
                        psq = ps_s.tile([P, TQ], F32, name="psq", tag="s")
                        for dt_i in range(DT):
                            nc.tensor.matmul(
                                psq[:],
                                wq_sb[:, dt_i, g * P:(g + 1) * P],
                                xT_sb[:, tqc, dt_i, :],
                                start=(dt_i == 0),
                                stop=(dt_i == DT - 1),
                            )
                        nc.vector.tensor_copy(qT_sb[:, g, sl], psq[:])

                # ---- main schedule: tiles run in DESCENDING tq order — the
                # largest tile (most exp backlog for the ACT engine) goes
                # first, hiding the remaining input DMA and the qproj of
                # later tiles; the smallest tile drains last. qproj for the
                # next tile is issued between the two attention calls so it
                # hides inside the ACT-bound phase.
                order = list(range(NTQ - 1, -1, -1))
                qproj(order[0])
                for idx, tqc in enumerate(order):
                    nxt = order[idx + 1] if idx + 1 < NTQ else None
                    if nxt is not None:
                        load_stage(nxt)
                    attention(0, tqc)
                    if idx == 0:
                        nc.sync.dma_start(xr_sb[:], xres[:])
                    if nxt is not None:
                        qproj(nxt)
                    attention(1, tqc)
                while pending:
                    epilogue(pending.popleft())

            if repeat == 1:
                body()
            else:
                tc.For_i_unrolled(0, repeat, 1, body, max_unroll=1)

    nc.compile()
    return nc


def prep_in_maps(x, k, v, Wq, variant: str = VARIANT):
    """Build the 8 per-core input maps from full inputs (host-side numpy)."""
    x = np.asarray(x, dtype=np.float32)
    k = np.asarray(k, dtype=np.float32)
    v = np.asarray(v, dtype=np.float32)
    Wq = np.asarray(Wq, dtype=np.float32)

    # maskT[r, c] = 1 where key r is visible to query c within the diagonal
    # 128x128 sub-block (r <= c).
    r_idx = np.arange(P)[:, None]
    c_idx = np.arange(P)[None, :]
    maskT_np = (r_idx <= c_idx).astype(np.float32)
    ident = np.eye(P, dtype=np.float32)

    in_maps = []
    for c in range(N_CORES):
        b = c // (N_CORES // B)
        grp = c % (N_CORES // B)
        heads = slice(HPC * grp, HPC * (grp + 1))
        cols = slice(EPC * grp, EPC * (grp + 1))

        xT_c = np.ascontiguousarray(
            x[b].T.reshape(DT, P, NTQ, TQ).transpose(1, 2, 0, 3)
        )                                               # [P, NTQ, DT, TQ]
        wqT_c = np.ascontiguousarray(
            Wq[cols, :].T.reshape(DT, P, EPC).transpose(1, 0, 2)
        )                                               # [P, DT, EPC]
        kT_c = np.zeros((P, EG, T), dtype=np.float32)
        for lh in range(HPC):
            kT_c[DH * (lh % 2):DH * (lh % 2) + DH, lh // 2, :] = \
                k[b, HPC * grp + lh].T
        vv = v[b, heads]                                # [HPC, T, DH]
        vO_c = np.ones((P, NTKB, HPC, DH + 1), dtype=np.float32)
        vO_c[:, :, :, :DH] = vv.reshape(HPC, NTKB, P, DH).transpose(2, 1, 0, 3)
        xres_c = np.ascontiguousarray(
            x[b][:, cols].reshape(NTKB, P, EPC).transpose(1, 0, 2)
        )
        in_maps.append({
            "xT": _host_cast(xT_c, variant),
            "wqT": _host_cast(wqT_c, variant),
            "kT": _host_cast(kT_c, variant),
            "vO": _host_cast(vO_c, variant),
            "xres": _host_cast(xres_c, variant),
            "maskT": _host_cast(maskT_np, variant),
            "ident": ident,
        })
    return in_maps


def gather_output(results):
    """Assemble full [B, T, D] output from 8 per-core [P, NTQ, 4, EPC]
    slices (tq = tqt*512 + j*128 + p)."""
    y = np.empty((B, T, D), dtype=np.float32)
    for c in range(N_CORES):
        b = c // (N_CORES // B)
        grp = c % (N_CORES // B)
        yc = results[c]["y"]            # [P, NTQ, HPC, 4, DH]
        y[b, :, EPC * grp:EPC * (grp + 1)] = (
            yc.transpose(1, 3, 0, 2, 4).reshape(T, EPC)
        )
    return y


_NC_CACHE = {}


def kernel(x, k, v, Wq):
    key = (VARIANT, 1)
    if key not in _NC_CACHE:
        _NC_CACHE[key] = build_nc(VARIANT, repeat=1)
    nc = _NC_CACHE[key]
    in_maps = prep_in_maps(x, k, v, Wq, VARIANT)
    res = run_bass_kernel_spmd(nc, in_maps, core_ids=list(range(N_CORES)))
    return gather_output(res.results)


# revision 52
# speedup vs baseline: 1.2342x; 1.2342x over previous
"""Trainium2 Bass kernel for nn_DecoderHead (B=2, T=2048, D=1024, H=16, DH=64).

y = x + softmax_causal((x @ Wq.T) split to heads @ k^T / sqrt(D)) @ v

Sharding: 8 cores = 2 (batch) x 4 (head groups of 4 heads). Each core computes
its batch's q-projection for its 256 output features (Wq column-sharded by
head), causal attention for its 4 heads, adds the residual slice, and writes a
[T, 256] slice; the host concatenates slices (the all-gather over the
head-split d dim is a free host-side assembly).

Per-core dataflow (all matmul contractions on the PE partition axis; bf16
operands by default — matmul column rate is the same as fp32r but DMA and
DVE elementwise costs halve/quarter):
  qT[e, t]   = sum_d WqT[d, e] * xT[d, t]         (q projection, transposed)
  sT[tk, tq] = sum_dh kT_h[dh, tk] * qT_h[dh, tq] (scores, transposed; two
               heads run concurrently in distinct PE row-groups since DH=64)
  eT         = exp(sT / 32) * tri_mask            (ACT exp; both exp and the
               downstream PV matmuls are sliced to skip fully-masked query
               columns of diagonal key blocks; the triangle mask-mul touches
               only the [128,128] diagonal sub-block)
  oT[dh', tq]= sum_tk vO[tk, dh'] * eT[tk, tq]    (vO = [v | ones]; row 64
                                                   accumulates the denominator)
  y[tq, dh]  = transpose(oT) / denom + x_res      (PE transpose into one PSUM
                                                   bank, fused DVE epilogue)

Schedule is tq-tile-major and fully interleaved: load stage c+1, project q for
tile c, then run attention for tile c — attention for tile 0 starts after
~2.5 MB of DMA and overlaps the remaining loads.
"""

import os
from collections import deque

import numpy as np

import concourse.bass as bass
import concourse.mybir as mybir
import concourse.tile as tile
from concourse import bacc
from concourse.alu_op_type import AluOpType
from concourse.bass_utils import run_bass_kernel_spmd

# Problem shape (hardcoded per the harness contract).
B, T, D, H = 2, 2048, 1024, 16
DH = D // H          # 64
N_CORES = 8
HPC = H // (N_CORES // B)   # heads per core = 4
EPC = HPC * DH       # output features per core = 256
P = 128              # SBUF partitions
TQ = 512             # query-tile width (matmul moving-dim)
NTQ = T // TQ        # 4
NTKB = T // P        # 16 key blocks of 128
DT = D // P          # 8 contraction tiles for the q projection
EG = EPC // P        # 2 head-pair groups of 128 e-rows
SCALE = 1.0 / np.sqrt(np.float32(D))   # 1/32 (reference scales by sqrt(d))

F32 = mybir.dt.float32
I16 = mybir.dt.int16
BF16 = mybir.dt.bfloat16

# Schraudolph-style exp on the DVE: intN(s*KS + bias) bitcast as a float
# approximates exp(s*SCALE) (exponent exact, mantissa linearly interpolated,
# ~±3% sawtooth — softmax normalization cancels it to ~0.3% output error).
# The bias TILE folds the causal mask: masked entries get a tiny exponent
# (~2^-95, i.e. weight 0). Diagonal score blocks use this path so the ACT
# engine (the exp bottleneck) only handles off-diagonal blocks.
def _schrau_consts(variant):
    if variant == "bf16":
        # int16 bitcast as bf16: 2^7 exponent scale
        k = float(SCALE) * float(np.log2(np.e)) * 128.0
        return mybir.dt.int16, k, 127.0 * 128.0 - 7.41, 4096.0
    # int32 bitcast as fp32/fp32r: 2^23 exponent scale
    k = float(SCALE) * float(np.log2(np.e)) * float(1 << 23)
    return mybir.dt.int32, k, float(127 * (1 << 23) - 486411), 260.0e6

# Matmul operand dtype: fp32r (default; fp32 w/ 11-bit mantissa, full PE
# rate at moving>=256, no separate Ldweights on HW), bf16 (half DMA but HW
# pays explicit weight loads — measured slower), fp32 (exact, 1/4 rate).
VARIANT = os.environ.get("DH_VARIANT", "fp32r")


def _mm_dt(variant):
    return {
        "fp32": mybir.dt.float32,
        "fp32r": mybir.dt.float32r,
        "bf16": mybir.dt.bfloat16,
    }[variant]


def _np_round_fp32r(a: np.ndarray) -> np.ndarray:
    """Round fp32 to the fp32r value set: 11-bit mantissa, RNE, low 12 bits 0."""
    u = a.astype(np.float32).view(np.uint32)
    lsb = (u >> np.uint32(12)) & np.uint32(1)
    r = (u + np.uint32(0x7FF) + lsb) & np.uint32(0xFFFFF000)
    return r.view(np.float32)


def _host_cast(a: np.ndarray, variant: str) -> np.ndarray:
    a = np.ascontiguousarray(a, dtype=np.float32)
    if variant == "fp32r":
        return _np_round_fp32r(a)
    if variant == "bf16":
        import ml_dtypes
        return a.astype(ml_dtypes.bfloat16)
    return a


def build_nc(variant: str = VARIANT, repeat: int = 1):
    """Build the per-core SPMD Bass program. `repeat` wraps the body in a
    hardware loop (timing only)."""
    mdt = _mm_dt(variant)
    nc = bacc.Bacc(
        "TRN2", target_bir_lowering=False, debug=False, num_devices=N_CORES
    )

    # Layouts are partition-major so every DMA lands as 128 descriptors of
    # multi-KB contiguous runs (the SP sequencer pays ~7ns/descriptor).
    xT = nc.dram_tensor("xT", [P, NTQ, DT, TQ], mdt, kind="ExternalInput").ap()
    wqT = nc.dram_tensor("wqT", [P, DT, EPC], mdt, kind="ExternalInput").ap()
    kT = nc.dram_tensor("kT", [P, EG, T], mdt, kind="ExternalInput").ap()
    vO = nc.dram_tensor("vO", [P, NTKB, HPC, DH + 1], mdt, kind="ExternalInput").ap()
    xres = nc.dram_tensor("xres", [P, T // P, EPC], mdt, kind="ExternalInput").ap()
    maskT = nc.dram_tensor("maskT", [P, P], mdt, kind="ExternalInput").ap()
    ident = nc.dram_tensor("ident", [P, P], F32, kind="ExternalInput").ap()
    y = nc.dram_tensor("y", [P, NTQ, HPC, 4, DH], F32,
                       kind="ExternalOutput").ap()

    with tile.TileContext(nc) as tc:
        with (
            tc.tile_pool(name="const", bufs=1) as cpool,
            tc.tile_pool(name="xq", bufs=1) as xqpool,
            tc.tile_pool(name="work", bufs=6) as wpool,
            tc.tile_pool(name="epi", bufs=2) as epool,
            tc.tile_pool(name="ps_s", bufs=3, space="PSUM") as ps_s,
            tc.tile_pool(name="ps_o", bufs=2, space="PSUM") as ps_o,
        ):
            idt, ks_k, bs_v, bs_m = _schrau_consts(variant)

            def body(_iv=None):
                # ---- tiles -------------------------------------------------
                id_sb = cpool.tile([P, P], F32, name="id_sb", tag="id_sb")
                mk_sb = cpool.tile([P, P], mdt, name="mk_sb", tag="mk_sb")
                wq_sb = xqpool.tile([P, DT, EPC], mdt, name="wq_sb", tag="wq_sb")
                xT_sb = xqpool.tile([P, NTQ, DT, TQ], mdt, name="xT_sb",
                                    tag="xT_sb")
                kT_sb = cpool.tile([P, EG, T], mdt, name="kT_sb", tag="kT_sb")
                vO_sb = cpool.tile([P, NTKB, HPC, DH + 1], mdt, name="vO_sb",
                                   tag="vO_sb")
                xr_sb = cpool.tile([P, T // P, EPC], mdt, name="xr_sb",
                                   tag="xr_sb")
                qT_sb = xqpool.tile([P, EG, T], mdt, name="qT_sb", tag="qT_sb")

                # ---- loads ------------------------------------------------
                # xT streams per-stage (qproj for tile c needs stage c); all
                # other inputs are single whole-tensor DMAs (128 descriptors
                # each) issued upfront and overlapped by the DMA engines.
                nc.sync.dma_start(id_sb[:], ident[:])
                nc.sync.dma_start(wq_sb[:], wqT[:])

                def load_stage(c):
                    nc.sync.dma_start(xT_sb[:, c], xT[:, c])

                load_stage(NTQ - 1)
                nc.sync.dma_start(mk_sb[:], maskT[:])
                # Quarter the big constants so the first attention pairs
                # only wait on the head of each stream.
                for q in range(4):
                    tsl = slice(q * (T // 4), (q + 1) * (T // 4))
                    nc.sync.dma_start(kT_sb[:, :, tsl], kT[:, :, tsl])
                    nc.sync.dma_start(
                        vO_sb[:, 4 * q:4 * (q + 1)], vO[:, 4 * q:4 * (q + 1)]
                    )

                # Warm-up while stage-0 DMA streams: prime the ACT exp table
                # and keep PE busy so the HAM clock-gate opens (dummy work on
                # the identity tile; results unused).
                warm_et = wpool.tile([P, P], F32, name="warm_et", tag="warm")
                psw = ps_o.tile([P, P], F32, name="psw", tag="o")
                for w in range(12):
                    nc.tensor.matmul(
                        psw[:], id_sb[:], id_sb[:], start=True, stop=True,
                    )
                nc.scalar.activation(
                    warm_et[:], psw[:],
                    mybir.ActivationFunctionType.Exp, scale=0.01,
                )

                pending = deque()

                def epilogue_start(h, tqt, pso_t):
                    oT = epool.tile([DH + 1, TQ], F32, name="oT", tag="oT",
                                    bufs=4)
                    nc.vector.tensor_copy(oT[:], pso_t[:])
                    return (h, tqt, oT)

                def epilogue(state):
                    h, tqt, oT = state
                    ysb = epool.tile([P, 4, DH], F32, name="ysb", tag="ysb",
                                     bufs=4)
                    pst = ps_o.tile([P, 4, DH + 1], F32, name="pst", tag="o")
                    for j in range(4):
                        nc.tensor.transpose(
                            pst[:, j, :],
                            oT[:, j * P:(j + 1) * P],
                            id_sb[0:DH + 1, 0:DH + 1],
                        )
                    rc = epool.tile([P, 4], F32, name="rc", tag="rc", bufs=4)
                    nc.vector.reciprocal(rc[:], pst[:, :, DH])
                    for j in range(4):
                        nc.vector.scalar_tensor_tensor(
                            ysb[:, j, :],
                            pst[:, j, 0:DH],
                            rc[:, j:j + 1],
                            xr_sb[:, 4 * tqt + j, h * DH:(h + 1) * DH],
                            AluOpType.mult,
                            AluOpType.add,
                        )
                    nc.sync.dma_start(y[:, tqt, h], ysb[:])

                def attention(hp, tqt):
                    g = hp
                    ntk = 4 * (tqt + 1)
                    npairs = ntk // 2
                    tq0 = tqt * TQ

                    def start_col(tkb):
                        """First non-fully-masked query column (within the
                        tile) for key block tkb."""
                        m = tkb - 4 * tqt
                        return P * m if m > 0 else 0

                    pso2 = [
                        ps_o.tile([DH + 1, TQ], F32, name=f"pso{i}", tag="o")
                        for i in range(2)
                    ]

                    def emit_pv(p_et2, p_pair, last=False):
                        for u in range(2):
                            tkb = 2 * p_pair + u
                            s0 = start_col(tkb)
                            for i in range(2):
                                nc.tensor.matmul(
                                    pso2[i][:, s0:],
                                    vO_sb[:, tkb, 2 * hp + i, :],
                                    p_et2[i][:, u, s0:],
                                    start=(tkb == 0),
                                    stop=(last and u == 1),
                                    skip_group_check=True,
                                )

                    prev = None
                    for pair in range(npairs):
                        diag = 2 * pair >= 4 * tqt
                        et2 = []
                        pssc2 = [
                            ps_s.tile([P, 2, TQ], F32, name=f"pssc{i}", tag="s")
                            for i in range(2)
                        ]
                        for i in range(2):
                            for u in range(2):
                                tkb = 2 * pair + u
                                s0 = start_col(tkb)
                                bp = DH * i
                                nc.tensor.matmul(
                                    pssc2[i][:, u, s0:],
                                    kT_sb[bp:bp + DH, g,
                                          tkb * P:(tkb + 1) * P],
                                    qT_sb[bp:bp + DH, g, tq0 + s0:tq0 + TQ],
                                    start=True,
                                    stop=True,
                                )
                        for i in range(2):
                            et = wpool.tile([P, 2, TQ], mdt,
                                            name=f"et{i}", tag="et")
                            if diag:
                                for u in range(2):
                                    tkb = 2 * pair + u
                                    s0 = start_col(tkb)
                                    nc.scalar.activation(
                                        et[:, u, s0:], pssc2[i][:, u, s0:],
                                        mybir.ActivationFunctionType.Exp,
                                        scale=float(SCALE),
                                    )
                                    nc.vector.tensor_mul(
                                        et[:, u, s0:s0 + P],
                                        et[:, u, s0:s0 + P],
                                        mk_sb[:],
                                    )
                            else:
                                nc.scalar.activation(
                                    et[:], pssc2[i][:],
                                    mybir.ActivationFunctionType.Exp,
                                    scale=float(SCALE),
                                )
                            et2.append(et)
                        if prev is not None:
                            emit_pv(*prev)
                        prev = (et2, pair)
                        if pending and pair < 2:
                            epilogue(pending.popleft())
                    emit_pv(*prev, last=True)
                    for i in range(2):
                        pending.append(epilogue_start(2 * hp + i, tqt, pso2[i]))

                def qproj(tqc):
                    sl = bass.ts(tqc, TQ)
                    for g in range(EG):
                        psq = ps_s.tile([P, TQ], F32, name="psq", tag="s")
                        for dt_i in range(DT):
                            nc.tensor.matmul(
                                psq[:],
                                wq_sb[:, dt_i, g * P:(g + 1) * P],
                                xT_sb[:, tqc, dt_i, :],
                                start=(dt_i == 0),
                                stop=(dt_i == DT - 1),
                            )
                        nc.vector.tensor_copy(qT_sb[:, g, sl], psq[:])

                # ---- main schedule: tiles run in DESCENDING tq order — the
                # largest tile (most exp backlog for the ACT engine) goes
                # first, hiding the remaining input DMA and the qproj of
                # later tiles; the smallest tile drains last. qproj for the
                # next tile is issued between the two attention calls so it
                # hides inside the ACT-bound phase.
                order = list(range(NTQ - 1, -1, -1))
                qproj(order[0])
                for idx, tqc in enumerate(order):
                    nxt = order[idx + 1] if idx + 1 < NTQ else None
                    if nxt is not None:
                        load_stage(nxt)
                    attention(0, tqc)
                    if idx == 0:
                        nc.sync.dma_start(xr_sb[:], xres[:])
                    if nxt is not None:
                        qproj(nxt)
                    attention(1, tqc)
                while pending:
                    epilogue(pending.popleft())

            if repeat == 1:
                body()
            else:
                tc.For_i_unrolled(0, repeat, 1, body, max_unroll=1)

    nc.compile()
    return nc


def prep_in_maps(x, k, v, Wq, variant: str = VARIANT):
    """Build the 8 per-core input maps from full inputs (host-side numpy)."""
    x = np.asarray(x, dtype=np.float32)
    k = np.asarray(k, dtype=np.float32)
    v = np.asarray(v, dtype=np.float32)
    Wq = np.asarray(Wq, dtype=np.float32)

    # maskT[r, c] = 1 where key r is visible to query c within the diagonal
    # 128x128 sub-block (r <= c).
    r_idx = np.arange(P)[:, None]
    c_idx = np.arange(P)[None, :]
    maskT_np = (r_idx <= c_idx).astype(np.float32)
    ident = np.eye(P, dtype=np.float32)

    in_maps = []
    for c in range(N_CORES):
        b = c // (N_CORES // B)
        grp = c % (N_CORES // B)
        heads = slice(HPC * grp, HPC * (grp + 1))
        cols = slice(EPC * grp, EPC * (grp + 1))

        xT_c = np.ascontiguousarray(
            x[b].T.reshape(DT, P, NTQ, TQ).transpose(1, 2, 0, 3)
        )                                               # [P, NTQ, DT, TQ]
        wqT_c = np.ascontiguousarray(
            Wq[cols, :].T.reshape(DT, P, EPC).transpose(1, 0, 2)
        )                                               # [P, DT, EPC]
        kT_c = np.zeros((P, EG, T), dtype=np.float32)
        for lh in range(HPC):
            kT_c[DH * (lh % 2):DH * (lh % 2) + DH, lh // 2, :] = \
                k[b, HPC * grp + lh].T
        vv = v[b, heads]                                # [HPC, T, DH]
        vO_c = np.ones((P, NTKB, HPC, DH + 1), dtype=np.float32)
        vO_c[:, :, :, :DH] = vv.reshape(HPC, NTKB, P, DH).transpose(2, 1, 0, 3)
        xres_c = np.ascontiguousarray(
            x[b][:, cols].reshape(NTKB, P, EPC).transpose(1, 0, 2)
        )
        in_maps.append({
            "xT": _host_cast(xT_c, variant),
            "wqT": _host_cast(wqT_c, variant),
            "kT": _host_cast(kT_c, variant),
            "vO": _host_cast(vO_c, variant),
            "xres": _host_cast(xres_c, variant),
            "maskT": _host_cast(maskT_np, variant),
            "ident": ident,
        })
    return in_maps


def gather_output(results):
    """Assemble full [B, T, D] output from 8 per-core [P, NTQ, 4, EPC]
    slices (tq = tqt*512 + j*128 + p)."""
    y = np.empty((B, T, D), dtype=np.float32)
    for c in range(N_CORES):
        b = c // (N_CORES // B)
        grp = c % (N_CORES // B)
        yc = results[c]["y"]            # [P, NTQ, HPC, 4, DH]
        y[b, :, EPC * grp:EPC * (grp + 1)] = (
            yc.transpose(1, 3, 0, 2, 4).reshape(T, EPC)
        )
    return y


_NC_CACHE = {}


def kernel(x, k, v, Wq):
    key = (VARIANT, 1)
    if key not in _NC_CACHE:
        _NC_CACHE[key] = build_nc(VARIANT, repeat=1)
    nc = _NC_CACHE[key]
    in_maps = prep_in_maps(x, k, v, Wq, VARIANT)
    res = run_bass_kernel_spmd(nc, in_maps, core_ids=list(range(N_CORES)))
    return gather_output(res.results)
